# revision 1
# baseline (speedup 1.0000x reference)
"""Trainium2 Bass kernel for nn_BaseTransformer (B=16, C=128, L=1024, H=8, dk=dv=32).

Sharding: pure data-parallel over batch — 8 cores x 2 batches each, no collectives.

Per-core algorithm (PE datapath in bf16 — fp32 matmuls stream at 1/4 rate on
this PE; PSUM accumulation and softmax normalization stay fp32):
  - QK projection: chunks of rows [q h0-3 | q h4-7 | k h0-3 | k h4-7], SCALE and
    q-bias folded in host-side (k bias dropped: softmax-invariant; v bias folded
    into the output bias via W_o @ b_v since softmax rows sum to 1).
  - v is projected TRANSPOSED (x^T @ Wv^T) so the PV matmul needs no transposes.
  - logits are computed transposed (S^T[t,s]) so softmax reduction happens via
    matmul against an all-ones stationary (denominator replicated over each
    head's 32 output partitions); exp runs on ScalarE straight out of PSUM.
  - attention matmuls are packed with tile_position (row-packing for K=32 QK,
    col-packing for M=32 PV and denominator) to use more of the PE array.
  - All compute ops keep out/in0/in1 at identical base partitions.

The exp of the logits (16.8M elements/core) is the critical path: every
element crosses PSUM->SBUF through ScalarE (1 elem/cycle @1.2GHz) or DVE
(1 elem/cycle @0.96GHz for fp32 src) — DMA and GpSimd have no PSUM route —
so the softmax exponentials are SPLIT across both engines: ScalarE runs
exact Exp on one head-pair tile while DVE runs a Schraudolph fast-exp on the
other — tensor_scalar computing int16(round(s*128/ln2 + (127-c)*128)) whose
int16 bits reinterpreted as bf16 are e^s with ~3% max elementwise error
(softmax ratios cancel most of it; end-to-end rel err stays ~2.7e-3).
Projection PSUM->SBUF moves are split across both engines too.

Scheduling (worth ~1.7x on HW over the naive emission order):
  - PE's queue is strict FIFO and sem-waits stall its sequencer, so any
    instruction emitted before ready work head-of-line-blocks it.  All
    emission is software-pipelined: QK(i+1) goes before PV(i) (which waits
    on exp(i)); each (g,j) phase's last PV + softmax-normalize drain inside
    the NEXT phase's first iteration; W_o of batch b drains inside the next
    attention call; projection pieces of the next batch/rep are spread one
    per two attention iterations as queue fillers.
  - PSUM (8 banks): depth-3 ring of [128,2,512] logit tiles (6 banks) +
    z/den accumulators (2 banks) exactly fills it; the depth-3 ring keeps
    the PE LDWEIGHTS+matmul latency out of the exp->exp chain.
"""

import os
import numpy as np

B, C, L = 16, 128, 1024
DK, DV, H = 32, 32, 8
SCALE = DK ** (-0.5)
NCORES = 8
BLOC = B // NCORES  # batches per core

_CACHE = {}

# bisect stages: proj < qkexp < pv < norm < full
_STAGES = ["proj", "qkexp", "pv", "norm", "full"]


def _stage():
    return os.environ.get("KSTAGE", "full")


def _stage_ge(s):
    return _STAGES.index(_stage()) >= _STAGES.index(s)


def _split_excess_waits(nc, mybir, cap=1):
    """This container's walrus rejects instructions carrying more than one
    sync-wait command ("Too many sync wait commands" in setupSyncWait), while
    Tile freely attaches several. Move all but `cap` waits of every
    instruction onto injected same-engine NoOps placed immediately before it
    (same block order == same engine queue order, so semantics are identical:
    all waits still complete before the instruction issues)."""
    ctr = 0
    for f in nc.m.functions:
        for blk in f.blocks:
            out = []
            changed = False
            for ins in blk.instructions:
                si = ins.sync_info
                waits = list(si.on_wait) if si and si.on_wait else []
                eng = getattr(ins, "engine", None)
                if len(waits) > cap and eng is not None:
                    for w in waits[:-cap]:
                        nop = mybir.InstNoOp(name=f"I-wsplit-{ctr}")
                        ctr += 1
                        nop.engine = eng
                        nop.sync_info = mybir.SyncInfo(on_wait=[w], on_update=[])
                        out.append(nop)
                    ins.sync_info = mybir.SyncInfo(
                        on_wait=waits[-cap:], on_update=list(si.on_update or [])
                    )
                    changed = True
                out.append(ins)
            if changed:
                blk.instructions = out


def _build_nc():
    import concourse.bass as bass
    import concourse.tile as tile
    from concourse import mybir
    from contextlib import ExitStack

    f32 = mybir.dt.float32
    bf16 = mybir.dt.bfloat16
    nc = bass.Bass()

    x_d = nc.dram_tensor("x_sh", [BLOC, C, L], bf16, kind="ExternalInput")
    wqk_d = nc.dram_tensor("wqk", [C, 4, 128], bf16, kind="ExternalInput")
    bqk_d = nc.dram_tensor("bqk", [128, 2], f32, kind="ExternalInput")
    wv_d = nc.dram_tensor("wv", [C, 256], bf16, kind="ExternalInput")
    wo_d = nc.dram_tensor("wo", [128, 3, 128], bf16, kind="ExternalInput")
    bout_d = nc.dram_tensor("bout", [128, 1], f32, kind="ExternalInput")
    out_d = nc.dram_tensor("out_sh", [BLOC, C, L], f32, kind="ExternalOutput")

    Exp = mybir.ActivationFunctionType.Exp
    Ident = mybir.ActivationFunctionType.Identity
    mult = mybir.AluOpType.mult
    add = mybir.AluOpType.add
    i16 = mybir.dt.int16

    # Schraudolph fast-exp targeting bf16 bits via int16:
    #   int16(s*128/ln2 + (127 - C)*128) bitcast to bf16 ~= e^s
    FEXP_A = 128.0 / float(np.log(2.0))
    FEXP_C = float(os.environ.get("KFEXPC", "0.0435"))
    FEXP_B = 128.0 * (127.0 - FEXP_C)

    with tile.TileContext(nc) as tc, ExitStack() as ctx:
        consts = ctx.enter_context(tc.tile_pool(name="consts", bufs=1))
        xp = ctx.enter_context(tc.tile_pool(name="xp", bufs=2))
        qkp = ctx.enter_context(tc.tile_pool(name="qkp", bufs=2))
        vtp = ctx.enter_context(tc.tile_pool(name="vtp", bufs=2))
        stp = ctx.enter_context(tc.tile_pool(name="stp", bufs=int(os.environ.get("KSTP", "6"))))
        zfp = ctx.enter_context(tc.tile_pool(name="zfp", bufs=2))
        rbp = ctx.enter_context(tc.tile_pool(name="rbp", bufs=3))
        outp = ctx.enter_context(tc.tile_pool(name="outp", bufs=2))
        # PSUM (8 banks x 2KB): pbig = logits/proj tiles (3 bufs x 2 banks),
        # pacc = z/den accumulators (1 buf x 1 bank each name x 2 names).
        # Depth-3 logits pipeline keeps PE's LDWEIGHTS+matmul latency for
        # iter i+1 out of the exp(i)->exp(i+1) critical chain.
        nbig = int(os.environ.get("KNBIG", "3"))
        nacc = int(os.environ.get("KNACC", "1"))
        pbig = ctx.enter_context(tc.tile_pool(name="pbig", bufs=nbig, space="PSUM"))
        pacc = ctx.enter_context(tc.tile_pool(name="pacc", bufs=nacc, space="PSUM"))

        wqk_sb = consts.tile([C, 4, 128], bf16, name="wqk_sb")
        bqk_sb = consts.tile([128, 2], f32, name="bqk_sb")
        wv_sb = consts.tile([C, 256], bf16, name="wv_sb")
        wo_sb = consts.tile([128, 3, 128], bf16, name="wo_sb")
        bout_sb = consts.tile([128, 1], f32, name="bout_sb")
        ones_sb = consts.tile([128, 32], bf16, name="ones_sb")
        nc.sync.dma_start(out=wqk_sb, in_=wqk_d[:, :, :])
        nc.sync.dma_start(out=bqk_sb, in_=bqk_d[:, :])
        nc.sync.dma_start(out=wv_sb, in_=wv_d[:, :])
        nc.sync.dma_start(out=wo_sb, in_=wo_d[:, :, :])
        nc.sync.dma_start(out=bout_sb, in_=bout_d[:, :])
        nc.vector.memset(ones_sb, 1.0)

        denmerge = bool(int(os.environ.get("KDENMERGE", "0")))
        vexp = bool(int(os.environ.get("KVEXP", "1")))
        # KXTRA>0: that many i-iters per (g,j) send the DVE tile to ScalarE
        # instead; KXTRA<0: ScalarE's tile goes to DVE. Load-balance knob.
        xtra = int(os.environ.get("KXTRA", "1"))
        # KXTRA8: finer balance knob — per 8 consecutive DVE-candidate tiles,
        # how many go to ScalarE instead (0..8). KXTRA=n == KXTRA8=n (legacy
        # semantics were per-(g,j) i<n, same 1-in-8 rate).
        xtra8 = int(os.environ.get("KXTRA8", str(xtra)))
        psplit = bool(int(os.environ.get("KPSPLIT", "1")))
        ilv = bool(int(os.environ.get("KILV", "1")))
        kfill = bool(int(os.environ.get("KFILL", "1")))
        kpipe = bool(int(os.environ.get("KPIPE", "1")))
        kdiv = bool(int(os.environ.get("KDIV", "0")))
        kpend = int(os.environ.get("KPEND", "2"))
        kfsplit = bool(int(os.environ.get("KFSPLIT", "0")))
        kosb_s = bool(int(os.environ.get("KOSB", "0")))
        repeat = int(os.environ.get("KREPEAT", "1"))
        giter = [0]  # global attention-iteration counter (for balance knobs)
        proj0_done = [False]  # batch-0 proj pre-emitted by the previous rep
        pending_wo = [[]]  # W_o emission deferred into the next attention call
        S = {}
        for _rep in range(repeat):
          def _proj_pieces(b):
            """Projection for batch b as independently emittable pieces, so
            its PSUM->SBUF moves can be spread into attention iterations
            instead of landing as one clump in the engine queues."""
            vw = 64 if denmerge else 32

            def p_alloc():
                x_sb = xp.tile([C, L], bf16, name="x_sb")
                nc.sync.dma_start(out=x_sb, in_=x_d[b])
                S[b] = dict(
                    x_sb=x_sb,
                    qA=qkp.tile([128, L], bf16, name="qA"),
                    qB=qkp.tile([128, L], bf16, name="qB"),
                    kA=qkp.tile([128, L], bf16, name="kA"),
                    kB=qkp.tile([128, L], bf16, name="kB"),
                    vt=vtp.tile([128, 8, 8, vw], bf16, name="vt"),
                )

            def mk_qk(cch):
              def p_qk():
                tgt = S[b][("qA", "qB", "kA", "kB")[cch]]
                ps = pbig.tile([128, L], f32, name="pl")
                for jh in range(2):
                    nc.tensor.matmul(
                        out=ps[:, 512 * jh : 512 * jh + 512],
                        lhsT=wqk_sb[:, cch, :],
                        rhs=S[b]["x_sb"][:, 512 * jh : 512 * jh + 512],
                        start=True, stop=True,
                    )
                # split the PSUM->SBUF proj moves across both engines so
                # neither queues a long copy-only bubble
                s_side = cch in ((0, 3) if psplit else (0, 1, 2, 3))
                if cch < 2:
                    if s_side:
                        nc.scalar.activation(
                            out=tgt, in_=ps, func=Ident,
                            bias=bqk_sb[:, cch : cch + 1],
                        )
                    else:
                        nc.vector.tensor_scalar_add(
                            out=tgt, in0=ps, scalar1=bqk_sb[:, cch : cch + 1]
                        )
                elif s_side:
                    nc.scalar.copy(out=tgt, in_=ps)
                else:
                    nc.vector.tensor_copy(out=tgt, in_=ps)
              return p_qk

            def mk_v(gq):
              def p_v():
                # ---- V^T projection: vt[t, i, h, d] = v_h[d, 128 i + t]
                vt = S[b]["vt"]
                ps = pbig.tile([128, L], f32, name="pl")
                for q in range(4):
                    nc.tensor.matmul(
                        out=ps[:, 256 * q : 256 * q + 256],
                        lhsT=S[b]["x_sb"][:, 128 * (4 * gq + q) : 128 * (4 * gq + q) + 128],
                        rhs=wv_sb,
                        start=True, stop=True,
                    )
                vdst = vt[:, 4 * gq : 4 * gq + 4, :, 0:32]
                vsrc = ps.rearrange("p (a h d) -> p a h d", h=8, d=32)
                if gq == 0 or not psplit:
                    nc.scalar.copy(out=vdst, in_=vsrc)
                else:
                    nc.vector.tensor_copy(out=vdst, in_=vsrc)
                if denmerge and gq == 1:
                    nc.gpsimd.memset(vt[:, :, :, 32:64], 1.0)
              return p_v

            return [p_alloc, mk_qk(0), mk_qk(2), mk_v(0), mk_qk(1), mk_qk(3), mk_v(1)]

          def _proj(b):
            for p in _proj_pieces(b):
                p()

          def _attn(b, g, fillers=()):
            fillers = list(fillers)
            pend = []  # deferred PV emissions (software pipeline, depth KPEND)

            def drain(all=False):
                while pend and (all or len(pend) > kpend - 1):
                    pend.pop(0)()
            x_sb, qA, qB, kA, kB, vt = (S[b][k] for k in
                ("x_sb", "qA", "qB", "kA", "kB", "vt"))
            if g == 0:
                S[b]["zfA"] = zfp.tile([128, L], bf16, name="zfA")
                S[b]["zfB"] = zfp.tile([128, L], bf16, name="zfB")
            zfA, zfB = S[b]["zfA"], S[b]["zfB"]
            if True:
                q_t = (qA, qB)[g]
                k_t = (kA, kB)[g]
                zf = (zfA, zfB)[g]
                # Software-pipelined emission: QK(i+1) is emitted BEFORE
                # PV(i), and the last PV + normalize of each (g,j) phase is
                # deferred into the next phase's first iteration.  The PE
                # queue is strict FIFO, and PV(i) waits on exp(i) — emitting
                # it eagerly would head-of-line-block ready QK matmuls and
                # starve both exp engines.
                for j in range(2):
                    sj = slice(512 * j, 512 * j + 512)
                    if denmerge:
                        # comb[p?]: per head pair: [z_even | den_even | z_odd | den_odd]
                        combs = [pacc.tile([128, 512], f32, name="comb", bufs=4)
                                 for _ in range(2)]
                        zden = denb = None
                    else:
                        zden = pacc.tile([128, 512], f32, name="zden")
                        denb = pacc.tile([128, 512], f32, name="denb")
                        combs = None

                    def emit_pv(sts, i, vt=vt, g=g, combs=combs, zden=zden, denb=denb):
                        if denmerge:
                            for pp in range(2):
                                for hh in range(2):
                                    nc.tensor.matmul(
                                        out=combs[pp][64 * hh : 64 * hh + 64, :],
                                        lhsT=vt[:, i, 4 * g + 2 * pp + hh, :],
                                        rhs=sts[pp][:, hh, :],
                                        start=(i == 0), stop=(i == 7),
                                        tile_position=(0, 64 * hh),
                                        skip_group_check=True,
                                    )
                        else:
                            for hl in range(4):  # head-local index in group
                                st = sts[hl // 2]
                                mv = st[:, hl % 2, :]
                                nc.tensor.matmul(
                                    out=zden[32 * hl : 32 * hl + 32, :],
                                    lhsT=vt[:, i, 4 * g + hl, :],
                                    rhs=mv,
                                    start=(i == 0), stop=(i == 7),
                                    tile_position=(0, 32 * hl),
                                    skip_group_check=True,
                                )
                                nc.tensor.matmul(
                                    out=denb[32 * hl : 32 * hl + 32, :],
                                    lhsT=ones_sb,
                                    rhs=mv,
                                    start=(i == 0), stop=(i == 7),
                                    tile_position=(0, 32 * hl),
                                    skip_group_check=True,
                                )

                    def norm(zf=zf, sj=sj, combs=combs, zden=zden, denb=denb):
                        if denmerge:
                            for pp in range(2):
                                rb = rbp.tile([128, 512], f32, name="rb")
                                nc.vector.reciprocal(out=rb, in_=combs[pp])
                                for hh in range(2):
                                    h4 = (2 * pp + hh) % 4
                                    nc.vector.tensor_tensor(
                                        out=zf[32 * h4 : 32 * h4 + 32, sj],
                                        in0=combs[pp][64 * hh : 64 * hh + 32, :],
                                        in1=rb[64 * hh + 32 : 64 * hh + 64, :],
                                        op=mult,
                                    )
                        elif _stage_ge("norm"):
                            if kdiv:
                                nc.vector.tensor_tensor(
                                    out=zf[:, sj], in0=zden, in1=denb,
                                    op=mybir.AluOpType.divide,
                                )
                            else:
                                rb = rbp.tile([128, 512], f32, name="rb")
                                nc.vector.reciprocal(out=rb, in_=denb)
                                nc.vector.tensor_tensor(
                                    out=zf[:, sj], in0=zden, in1=rb, op=mult
                                )
                        else:
                            nc.vector.tensor_copy(out=zf[:, sj], in_=zden)

                    for i in range(8):
                        sts = []
                        for pp in range(2):  # head pairs within group
                            r0 = 64 * pp
                            pl = pbig.tile([128, 2, 512], f32, name="pl")
                            for hh in range(2):
                                rr = r0 + 32 * hh
                                nc.tensor.matmul(
                                    out=pl[:, hh, :],
                                    lhsT=k_t[rr : rr + 32, 128 * i : 128 * i + 128],
                                    rhs=q_t[rr : rr + 32, sj],
                                    start=True, stop=True,
                                    tile_position=(rr, 0),
                                )
                            st = stp.tile([128, 2, 512], bf16, name="st")
                            if vexp:
                                if pp == 1:
                                    on_dve = (giter[0] % 8) >= xtra8
                                else:
                                    on_dve = xtra8 < 0 and (giter[0] % 8) < -xtra8
                                giter[0] += pp
                            else:
                                on_dve = False
                            if on_dve:
                                nc.vector.tensor_scalar(
                                    out=st.bitcast(i16)[:, :, :], in0=pl,
                                    scalar1=FEXP_A, scalar2=FEXP_B,
                                    op0=mult, op1=add,
                                )
                            else:
                                nc.scalar.activation(out=st, in_=pl, func=Exp)
                            sts.append(st)
                        if kpipe:
                            if i == 7:
                                pend.append(lambda sts=sts, e=emit_pv, n=norm:
                                            (e(sts, 7), n()))
                            else:
                                pend.append(lambda sts=sts, i=i, e=emit_pv:
                                            e(sts, i))
                            drain()
                        else:
                            emit_pv(sts, i)
                            if i == 7:
                                norm()
                        if fillers and i % 2 == 1:
                            fillers.pop(0)()
            drain(all=True)
            for f in fillers:  # any leftover pieces
                f()

          def _wo(b):
            x_sb, zfA, zfB = (S[b][k] for k in ("x_sb", "zfA", "zfB"))
            # ---- output projection + residual projection + bias
            po = pbig.tile([128, L], f32, name="pl")
            for j in range(2):
                sj = slice(512 * j, 512 * j + 512)
                nc.tensor.matmul(out=po[:, sj], lhsT=wo_sb[:, 0, :], rhs=zfA[:, sj],
                                 start=True, stop=False)
                nc.tensor.matmul(out=po[:, sj], lhsT=wo_sb[:, 1, :], rhs=zfB[:, sj],
                                 start=False, stop=False)
                nc.tensor.matmul(out=po[:, sj], lhsT=wo_sb[:, 2, :], rhs=x_sb[:, sj],
                                 start=False, stop=True)
            o_sb = outp.tile([128, L], f32, name="o_sb")
            if kosb_s:
                nc.scalar.activation(out=o_sb, in_=po, func=Ident,
                                     bias=bout_sb[:, 0:1])
            else:
                nc.vector.tensor_scalar_add(out=o_sb, in0=po, scalar1=bout_sb[:, 0:1])
            nc.sync.dma_start(out=out_d[b], in_=o_sb)

          # Interleaved schedule: batch b+1's projection pieces are spread one
          # per two attention iterations of batch b (and the next rep's first
          # projection into the last batch's tail) so the proj PSUM->SBUF
          # moves fill engine gaps instead of forming an up-front bubble.
          if ilv:
            if not proj0_done[0]:
                _proj(0)
            proj0_done[0] = False
            for b in range(BLOC):
                fills = list(pending_wo[0])
                pending_wo[0] = []
                if b + 1 < BLOC:
                    if kfill:
                        pieces = fills + _proj_pieces(b + 1)
                        nsplit = len(pieces) // 2 if kfsplit else len(pieces)
                        _attn(b, 0, fillers=pieces[:nsplit])
                        _attn(b, 1, fillers=pieces[nsplit:])
                    else:
                        for f in fills:
                            f()
                        _attn(b, 0)
                        _proj(b + 1)
                        _attn(b, 1)
                elif _rep + 1 < repeat and kfill:
                    _attn(b, 0, fillers=fills)
                    _attn(b, 1, fillers=_proj_pieces(0))
                    proj0_done[0] = True
                else:
                    for f in fills:
                        f()
                    _attn(b, 0)
                    _attn(b, 1)
                if kfill and not (b + 1 == BLOC and _rep + 1 == repeat):
                    # defer W_o into the next attention call so its PE matmuls
                    # (which wait on this batch's normalize) don't head-of-line
                    # block the next phase's QK matmuls
                    pending_wo[0] = [lambda b=b: _wo(b)]
                else:
                    _wo(b)
          else:
            for b in range(BLOC):
                _proj(b)
            for b in range(BLOC):
                _attn(b, 0)
                _attn(b, 1)
            for b in range(BLOC):
                _wo(b)

    _split_excess_waits(nc, mybir)
    nc.finalize()
    return nc


def get_nc():
    if "nc" not in _CACHE:
        _CACHE["nc"] = _build_nc()
    return _CACHE["nc"]


def prep_weights(w_qkv, b_qkv, w_o, b_o, w_res, b_res):
    w_qkv = np.asarray(w_qkv, np.float32)
    b_qkv = np.asarray(b_qkv, np.float32)
    w_o = np.asarray(w_o, np.float32)
    b_o = np.asarray(b_o, np.float32)
    w_res = np.asarray(w_res, np.float32)
    b_res = np.asarray(b_res, np.float32)

    d = np.arange(32)
    qrows = np.concatenate([96 * h + d for h in range(H)])        # (256,)
    krows = np.concatenate([96 * h + 32 + d for h in range(H)])
    vrows = np.concatenate([96 * h + 64 + d for h in range(H)])

    Wq = w_qkv[qrows] * SCALE                                     # (256, C)
    Wk = w_qkv[krows]
    wqk = np.stack([Wq[:128].T, Wq[128:].T, Wk[:128].T, Wk[128:].T], axis=1)
    bqk = np.stack([b_qkv[qrows[:128]], b_qkv[qrows[128:]]], axis=1) * SCALE
    wv = np.ascontiguousarray(w_qkv[vrows].T)                     # (C, 256)
    wo = np.stack([w_o[:, :128].T, w_o[:, 128:].T, w_res.T], axis=1)
    bv = b_qkv[vrows]
    bout = (b_o + b_res + w_o @ bv)[:, None]

    import ml_dtypes
    bf = ml_dtypes.bfloat16
    return {
        "wqk": np.ascontiguousarray(wqk, bf),
        "bqk": np.ascontiguousarray(bqk, np.float32),
        "wv": np.ascontiguousarray(wv, bf),
        "wo": np.ascontiguousarray(wo, bf),
        "bout": np.ascontiguousarray(bout, np.float32),
    }


def make_in_maps(x, weights):
    import ml_dtypes
    x = np.ascontiguousarray(np.asarray(x).astype(ml_dtypes.bfloat16))
    return [
        dict(x_sh=np.ascontiguousarray(x[BLOC * i : BLOC * i + BLOC]), **weights)
        for i in range(NCORES)
    ]


class Runner:
    """Persistent PJRT executable for the SPMD bass program (axon path).

    Mirrors concourse.bass2jax.run_bass_via_pjrt's multi-core branch, but keeps
    the jitted callable so repeated executions don't re-trace/re-compile —
    needed both for a fast kernel() and for timing loops in test.py.
    """

    def __init__(self, nc=None, donate=True):
        import jax
        import concourse.mybir as mybir
        from concourse import bass2jax
        from jax.experimental.shard_map import shard_map
        from jax.sharding import Mesh, PartitionSpec

        if nc is None:
            nc = get_nc()
        bass2jax.install_neuronx_cc_hook()

        in_names, out_names, out_avals = [], [], []
        partition_name = (
            nc.partition_id_tensor.name if nc.partition_id_tensor else None
        )
        for alloc in nc.m.functions[0].allocations:
            if not isinstance(alloc, mybir.MemoryLocationSet):
                continue
            name = alloc.memorylocations[0].name
            if alloc.kind == "ExternalInput":
                if name != partition_name:
                    in_names.append(name)
            elif alloc.kind == "ExternalOutput":
                shape = tuple(alloc.tensor_shape)
                dtype = mybir.dt.np(alloc.dtype)
                out_avals.append(jax.core.ShapedArray(shape, dtype))
                out_names.append(name)
        n_params = len(in_names)
        n_outs = len(out_avals)
        all_in_names = list(in_names) + list(out_names)
        if partition_name is not None:
            all_in_names.append(partition_name)
        self.in_names = in_names
        self.out_names = out_names
        self.out_avals = out_avals

        donate_idx = tuple(range(n_params, n_params + n_outs)) if donate else ()

        def _body(*args):
            operands = list(args)
            if partition_name is not None:
                operands.append(bass2jax.partition_id_tensor())
            outs = bass2jax._bass_exec_p.bind(
                *operands,
                out_avals=tuple(out_avals),
                in_names=tuple(all_in_names),
                out_names=tuple(out_names),
                lowering_input_output_aliases=(),
                sim_require_finite=True,
                sim_require_nnan=True,
                nc=nc,
            )
            return tuple(outs)

        devices = jax.devices()[:NCORES]
        assert len(devices) == NCORES
        mesh = Mesh(np.asarray(devices), ("core",))
        in_specs = (PartitionSpec("core"),) * (n_params + n_outs)
        out_specs = (PartitionSpec("core"),) * n_outs
        self.sharded = jax.jit(
            shard_map(_body, mesh=mesh, in_specs=in_specs, out_specs=out_specs,
                      check_rep=False),
            donate_argnums=donate_idx,
            keep_unused=True,
        )
        self.mesh = mesh

    def prep(self, in_maps):
        return [
            np.concatenate([np.asarray(m[name]) for m in in_maps], axis=0)
            for name in self.in_names
        ]

    def zeros(self):
        return [
            np.zeros((NCORES * a.shape[0], *a.shape[1:]), a.dtype)
            for a in self.out_avals
        ]

    def call_async(self, concat_in):
        return self.sharded(*concat_in, *self.zeros())

    def __call__(self, in_maps):
        outs = self.call_async(self.prep(in_maps))
        arr = np.asarray(outs[0])
        return arr.reshape(NCORES, *self.out_avals[0].shape)


def get_runner():
    if "runner" not in _CACHE:
        _CACHE["runner"] = Runner()
    return _CACHE["runner"]


def run(x, weights, **kw):
    runner = get_runner()
    per_core = runner(make_in_maps(x, weights))
    out = per_core.reshape(B, C, L)
    return out, None


def kernel(x, w_qkv, b_qkv, w_o, b_o, w_res, b_res):
    weights = prep_weights(w_qkv, b_qkv, w_o, b_o, w_res, b_res)
    out, _ = run(x, weights)
    return out

